# revision 19
# baseline (speedup 1.0000x reference)
"""Causal self-attention (RoPE + parameter-free RMSNorm on Q/K) — bf16 kernel.

Sharding: 8 cores = 4 batch x 2 head-groups (8 heads each). Each core computes
its batch element's attention for its 8 heads plus the transposed partial
output projection; the host sums the two head-group partials per batch.

v3 vs v2:
  - Single ACT table set (natural_log_exp_and_others): rms factors via
    rsqrt(x) = exp(-0.5*ln(x)) instead of Sqrt + DVE reciprocal, so the
    Exp tables never reload between phases.
  - Weight prefetch: pass p+1's Wq/Wk/Wv load during pass p's attention,
    Wo during the out-projection, so no DMA stall at body start.
  - Scores matmuls skip the masked-out columns on diagonal chunks.
  - Softmax denominator broadcast: two reciprocals land on partitions
    64/65, one K=2 matmul broadcasts both heads, staging copy on DVE
    (ACT does only exp during attention).

Per-core device layout (D=64, 8 heads):
  Q^T / K^T stored as [128, 4, T] bf16: col j = 128*cc + p,
     cc = 2*(h//4) + half, p = 32*(h%4) + r,  (d = 32*half + r)
  V stored with a ones column per head: [128, T//128, 8*66] bf16; the ones
  column makes the PV matmul also accumulate the softmax denominator. Even
  heads put it at col 64 (-> PSUM row 64), odd heads at col 65 (-> row 65),
  so a head-pair's denominators land on adjacent partitions and one K=2
  matmul broadcasts both reciprocals.
  Scores computed transposed: S^T[tk, tq] per head via K=32 row-tiled matmuls;
  softmax runs without max-subtraction (RMS-normed q,k bound |s| <= 8);
  the denominator division folds in before the output projection.
"""

import sys

import numpy as np

for _p in ("/opt/trn_rl_repo",):
    if _p not in sys.path:
        sys.path.insert(0, _p)

import ml_dtypes

import concourse.bass as bass
import concourse.mybir as mybir
import concourse.tile as tile
from concourse import bacc

F32 = mybir.dt.float32
BF16 = mybir.dt.bfloat16
AX = mybir.AluOpType
ACTF = mybir.ActivationFunctionType
BFNP = ml_dtypes.bfloat16

D = 64
NH = 8          # heads per core
CH = NH * D     # 512 head channels per core
EPS = float(np.finfo(np.float32).eps)


def qk_col_perm():
    """perm[j] = plain column (64*h + d) stored at device column j."""
    perm = np.zeros(CH, dtype=np.int64)
    for h in range(NH):
        for half in range(2):
            for r in range(32):
                j = 128 * (2 * (h // 4) + half) + 32 * (h % 4) + r
                perm[j] = 64 * h + 32 * half + r
    return perm


# ----------------------------------------------------------------------
# Const blob layout (bf16). Each entry: name -> (shape, partition dim).
# Packed on host in C order with the partition dim first, so the device
# can slice CONST[0, off:off+size].rearrange("(p n) -> p n", p=P).
# ----------------------------------------------------------------------
def blob_layout(T, CIN, COUT):
    KC = CIN // 128
    return [
        ("WQ", (128, KC * CH)),        # [ki, (ko m)]
        ("WK", (128, KC * CH)),
        ("WV", (128, KC * CH)),
        ("WO", (128, 4 * COUT)),       # [mi, (mo n)]
        ("COS", (128, T)),             # tiled 4x along partitions
        ("SIN", (128, T)),
        ("MASK", (128, 128)),          # [p, j] = p <= j
        ("SELA", (128, 8)),            # ssq reduction /64, heads 0-3
        ("SELB", (128, 8)),            # heads 4-7
        ("SELTA", (8, 128)),           # k rms broadcast, chunks 0,1
        ("SELTB", (8, 128)),           # chunks 2,3
        ("SELTAQ", (8, 128)),          # q rms broadcast (x 1/8), chunks 0,1
        ("SELTBQ", (8, 128)),
        ("SELRB", (128, 128)),         # denom broadcast: rows 64,65
        ("ONES", (128, 128)),          # ones: V ones-columns + denom bcast
    ]


def make_blob_consts(T, cos_t, sin_t):
    """Host-side constant arrays (bf16) keyed by blob entry name.
    cos_t/sin_t: [T, 32] fp32 RoPE tables."""
    cosT = np.ascontiguousarray(cos_t.T)  # [32, T]
    sinT = np.ascontiguousarray(sin_t.T)
    COS = np.tile(cosT, (4, 1))
    SIN = np.tile(sinT, (4, 1))
    p = np.arange(128)[:, None]
    j = np.arange(128)[None, :]
    MASK = (p <= j).astype(np.float32)
    SELA = np.zeros((128, 8), dtype=np.float32)
    SELB = np.zeros((128, 8), dtype=np.float32)
    for pp in range(128):
        SELA[pp, pp // 32] = 1.0 / 64.0
        SELB[pp, 4 + pp // 32] = 1.0 / 64.0
    SELTA = (64.0 * SELA.T).copy()          # entries 1
    SELTB = (64.0 * SELB.T).copy()
    SELTAQ = SELTA / 8.0                    # folds the 1/sqrt(D) scale
    SELTBQ = SELTB / 8.0
    SELRB = np.zeros((128, 128), dtype=np.float32)
    SELRB[64, 0:64] = 1.0
    SELRB[65, 64:128] = 1.0
    ONES = np.ones((128, 128), dtype=np.float32)
    return dict(COS=COS, SIN=SIN, MASK=MASK, SELA=SELA, SELB=SELB,
                SELTA=SELTA, SELTB=SELTB, SELTAQ=SELTAQ, SELTBQ=SELTBQ,
                SELRB=SELRB, ONES=ONES)


def pack_blob(T, CIN, COUT, named):
    """Pack named arrays (host dtype any float) into one bf16 blob [1, N]."""
    chunks = []
    for name, shape in blob_layout(T, CIN, COUT):
        a = np.asarray(named[name], dtype=np.float32)
        assert a.shape == shape, (name, a.shape, shape)
        chunks.append(np.ascontiguousarray(a).astype(BFNP).reshape(-1))
    return np.concatenate(chunks)[None, :]


def blob_offsets(T, CIN, COUT):
    offs = {}
    off = 0
    for name, shape in blob_layout(T, CIN, COUT):
        n = int(np.prod(shape))
        offs[name] = (off, shape)
        off += n
    return offs, off


def build_nc(T, CIN, COUT, repeat=1, phases="ABC", sub="qkvrsm"):
    """Build the Bass program. If repeat > 1, the whole computation runs
    `repeat` times in a device-side loop (for steady-state timing).
    phases: subset of "ABC" to emit (ablation/debug).
    sub: phase-A pieces: q/k/v projections, r=rope, s=ssq, m=rms-apply."""
    assert T % 512 == 0 and CIN % 128 == 0 and COUT % 512 == 0
    KC = CIN // 128        # c_in chunks
    NTB = T // 512         # projection t-blocks == tq blocks
    NQ = T // 512
    NKC = T // 128         # tk chunks
    NO8 = COUT // 128      # out-proj column chunks

    nc = bacc.Bacc()

    offs, blob_n = blob_offsets(T, CIN, COUT)
    # XT host-packed as [ki, tb, ko, tt] so each t-block load is one
    # contiguous 8KB-per-partition DMA.
    XT = nc.dram_tensor("XT", [128, (CIN // 128) * T], BF16,
                        kind="ExternalInput")
    CONST = nc.dram_tensor("CONST", [1, blob_n], BF16, kind="ExternalInput")
    OUTT = nc.dram_tensor("OUTT", [COUT, T], BF16, kind="ExternalOutput")

    def cslice(name):
        off, shape = offs[name]
        n = int(np.prod(shape))
        ap = CONST.ap()[0, off:off + n]
        return ap.rearrange("(p n) -> p n", p=shape[0])

    def xblock(tb):
        KC_ = CIN // 128
        sl = XT.ap()[:, tb * KC_ * 512:(tb + 1) * KC_ * 512]
        return sl.rearrange("p (k t) -> p k t", k=KC_)

    with tile.TileContext(nc) as tc:
        with (
            tc.tile_pool(name="consts", bufs=1) as cpool,
            tc.tile_pool(name="big", bufs=1) as big,
            tc.tile_pool(name="w", bufs=1) as wpool,
            tc.tile_pool(name="xtb", bufs=2) as xpool,
            tc.tile_pool(name="work", bufs=1) as work,
            tc.tile_pool(name="tmp", bufs=2) as tmp,
            tc.tile_pool(name="psa", bufs=2, space="PSUM") as psa,
            tc.tile_pool(name="psy", bufs=4, space="PSUM") as psy,
        ):
            # ---- constants (loaded once, outside the repeat loop) ----
            mask_sb = cpool.tile([128, 128], BF16, tag="mask")
            nc.sync.dma_start(out=mask_sb, in_=cslice("MASK"))
            sela_sb = cpool.tile([128, 8], BF16, tag="sela")
            nc.sync.dma_start(out=sela_sb, in_=cslice("SELA"))
            selb_sb = cpool.tile([128, 8], BF16, tag="selb")
            nc.sync.dma_start(out=selb_sb, in_=cslice("SELB"))
            selta_sb = cpool.tile([8, 128], BF16, tag="selta")
            nc.sync.dma_start(out=selta_sb, in_=cslice("SELTA"))
            seltb_sb = cpool.tile([8, 128], BF16, tag="seltb")
            nc.sync.dma_start(out=seltb_sb, in_=cslice("SELTB"))
            seltaq_sb = cpool.tile([8, 128], BF16, tag="seltaq")
            nc.sync.dma_start(out=seltaq_sb, in_=cslice("SELTAQ"))
            seltbq_sb = cpool.tile([8, 128], BF16, tag="seltbq")
            nc.sync.dma_start(out=seltbq_sb, in_=cslice("SELTBQ"))
            selrb_sb = cpool.tile([128, 128], BF16, tag="selrb")
            nc.sync.dma_start(out=selrb_sb, in_=cslice("SELRB"))
            ones_sb = cpool.tile([128, 64], BF16, tag="ones")
            nc.sync.dma_start(out=ones_sb, in_=cslice("ONES")[:, 0:64])
            cos_sb = cpool.tile([128, T], BF16, tag="cos")
            nc.sync.dma_start(out=cos_sb, in_=cslice("COS"))
            sin_sb = cpool.tile([128, T], BF16, tag="sin")
            nc.sync.dma_start(out=sin_sb, in_=cslice("SIN"))
            epsb = cpool.tile([8, 1], F32, tag="epsb")
            nc.vector.memset(epsb, EPS)

            khat = big.tile([128, 4, T], BF16, tag="khat")
            qhat = big.tile([128, 4, T], BF16, tag="qhat")
            vsb = big.tile([128, NKC, 8 * 66], BF16, tag="v")
            vsb4 = vsb.rearrange("p n (h e) -> p n h e", e=66)

            wq_sb = wpool.tile([128, KC, CH], BF16, tag="wa")
            wk_sb = wpool.tile([128, KC, CH], BF16, tag="wc")
            wv_sb = wpool.tile([128, KC, CH], BF16, tag="wb")
            wo_sb = wpool.tile([128, 4, COUT], BF16, tag="wo")

            def load_qkv_weights():
                nc.sync.dma_start(
                    out=wq_sb,
                    in_=cslice("WQ").rearrange("p (k m) -> p k m", k=KC))
                nc.sync.dma_start(
                    out=wk_sb,
                    in_=cslice("WK").rearrange("p (k m) -> p k m", k=KC))
                nc.sync.dma_start(
                    out=wv_sb,
                    in_=cslice("WV").rearrange("p (k m) -> p k m", k=KC))

            def load_wo():
                nc.sync.dma_start(
                    out=wo_sb,
                    in_=cslice("WO").rearrange("p (m n) -> p m n", m=4))

            # V ones-columns never overwritten: load once. Every head gets
            # ones at cols 64 AND 65, so its PV bank holds the denominator
            # on both partitions; partition-aligned reads then pick row 64
            # (even head) / rows 64-66 (odd head, row 64 overwritten).
            for _half in range(2):
                nc.sync.dma_start(
                    out=vsb4[:, 8 * _half:8 * _half + 8, :, 64:66],
                    in_=cslice("ONES")[:, 0:128].rearrange(
                        "p (n h e) -> p n h e", n=8, h=8))

            def project_qk(w_sb, xtb, dst, ts, ssq_t):
                """Project one 512-t block into dst[:, :, ts] (unscaled rope);
                accumulate the per-head sum of squares / 64 into ssq_t."""
                qpa = psa.tile([128, 2, 512], F32, tag="pa", name="qpa")
                qpb = psa.tile([128, 2, 512], F32, tag="pa", name="qpb")
                for cc in range(4):
                    qp_t = qpa if cc < 2 else qpb
                    for k in range(KC):
                        nc.tensor.matmul(
                            qp_t[:, cc % 2, :],
                            w_sb[:, k, 128 * cc:128 * (cc + 1)],
                            xtb[:, k, :],
                            start=(k == 0), stop=(k == KC - 1),
                        )
                # stage to SBUF bf16 on ACT so rope runs on DVE in 2x mode
                qs = tmp.tile([128, 4, 512], BF16, tag="qs", bufs=2)
                nc.scalar.activation(qs[:, 0:2, :], qpa, ACTF.Copy)
                nc.scalar.activation(qs[:, 2:4, :], qpb, ACTF.Copy)
                # unscaled rope into dst (scaled afterwards, once rms known)
                if "r" in sub:
                    u1 = qs[:, 0::2, :]
                    u2 = qs[:, 1::2, :]
                    cosb = cos_sb[:, None, ts].to_broadcast([128, 2, 512])
                    sinb = sin_sb[:, None, ts].to_broadcast([128, 2, 512])
                    e1 = tmp.tile([128, 2, 512], BF16, tag="r512", bufs=2)
                    e2 = tmp.tile([128, 2, 512], BF16, tag="r512", bufs=2)
                    nc.vector.tensor_mul(e1, u1, cosb)
                    nc.vector.tensor_mul(e2, u2, sinb)
                    nc.vector.tensor_add(dst[:, 0::2, ts], e1, e2)
                    e3 = tmp.tile([128, 2, 512], BF16, tag="r512", bufs=2)
                    e4 = tmp.tile([128, 2, 512], BF16, tag="r512", bufs=2)
                    nc.vector.tensor_mul(e3, u2, cosb)
                    nc.vector.tensor_mul(e4, u1, sinb)
                    nc.vector.tensor_sub(dst[:, 1::2, ts], e3, e4)
                else:
                    nc.vector.tensor_copy(out=dst[:, :, ts], in_=qs)
                if "s" not in sub:
                    return
                # per-head sum of squares (pre-rope == post-rope norms)
                qsq = tmp.tile([128, 4, 512], BF16, tag="qsq", bufs=2)
                nc.vector.tensor_mul(qsq, qs, qs)
                for cc in range(4):
                    nc.tensor.matmul(
                        ssq_t,
                        sela_sb if cc < 2 else selb_sb,
                        qsq[:, cc, :],
                        start=(cc == 0), stop=(cc == 3),
                    )

            def rms_apply(ssq_t, dst, ts, selts):
                """rr = 1/sqrt(ssq_t + eps) = exp(-0.5*ln(ssq_t + eps)),
                broadcast via PE and applied to dst[:, :, ts]."""
                lnt = tmp.tile([8, 512], F32, tag="lnt", bufs=2)
                nc.scalar.activation(lnt, ssq_t, ACTF.Ln, bias=epsb)
                rr = tmp.tile([8, 512], BF16, tag="rr", bufs=2)
                with nc.allow_low_precision(reason="bf16 rms factors"):
                    nc.scalar.activation(rr, lnt, ACTF.Exp, scale=-0.5)
                if "m" not in sub:
                    return
                for pr in range(2):
                    bq = psy.tile([128, 512], F32, tag="y", name=f"bq{pr}")
                    nc.tensor.matmul(
                        bq, selts[pr], rr,
                        start=True, stop=True,
                    )
                    bqs = tmp.tile([128, 512], BF16, tag="bqs", bufs=2)
                    nc.scalar.activation(bqs, bq, ACTF.Copy)
                    nc.vector.tensor_mul(
                        dst[:, 2 * pr:2 * pr + 2, ts],
                        dst[:, 2 * pr:2 * pr + 2, ts],
                        bqs[:, None, :].to_broadcast([128, 2, 512]),
                    )

            def body(prefetch):
                # ===== Phase A: Q-hat, K-hat, V (per t-block) =====
                for tb in range(NTB if "A" in phases else 0):
                    ts = slice(512 * tb, 512 * (tb + 1))
                    xtb = xpool.tile([128, KC, 512], BF16, tag="xtb")
                    nc.sync.dma_start(out=xtb, in_=xblock(tb))
                    ssq_q = psy.tile([8, 512], F32, tag="y", name="ssq_q")
                    ssq_k = psy.tile([8, 512], F32, tag="y", name="ssq_k")
                    if "q" in sub:
                        project_qk(wq_sb, xtb, qhat, ts, ssq_q)
                    if "k" in sub:
                        project_qk(wk_sb, xtb, khat, ts, ssq_k)
                    if "s" in sub:
                        if "q" in sub:
                            rms_apply(ssq_q, qhat, ts, (seltaq_sb, seltbq_sb))
                        if "k" in sub:
                            rms_apply(ssq_k, khat, ts, (selta_sb, seltb_sb))
                    for j in range(4 if "v" in sub else 0):
                        vp = psy.tile([128, 512], F32, tag="y",
                                      name=f"vp{tb}_{j}")
                        for k in range(KC):
                            nc.tensor.matmul(
                                vp,
                                xtb[:, k, 128 * j:128 * (j + 1)],
                                wv_sb[:, k, :],
                                start=(k == 0), stop=(k == KC - 1),
                            )
                        nc.scalar.activation(
                            vsb4[:, 4 * tb + j, :, 0:64],
                            vp.rearrange("p (h d) -> p h d", d=64), ACTF.Copy)

                # prefetch next pass's qkv weights during attention
                if prefetch:
                    load_qkv_weights()

                # ===== Phase B: per tq block: attention =====
                yhat = big.tile([128, 4, T], BF16, tag="yhat")
                for qb in range(NQ if "B" in phases else 0):
                    tqs = slice(512 * qb, 512 * (qb + 1))
                    for g in range(2):
                        ybank = [psy.tile([66, 512], F32, tag="y",
                                          name=f"y{qb}_{g}_{j_}")
                                 for j_ in range(4)]
                        nkc = 4 * (qb + 1)

                        def emit_pv(c, phs, lo, first, last):
                            for pj in range(2):
                                for e in range(2):
                                    j = 2 * pj + e
                                    hloc = 4 * g + j
                                    nc.tensor.matmul(
                                        ybank[j][:, lo:],
                                        vsb[:, c, 66 * hloc:66 * hloc + 66],
                                        phs[pj][:, e, lo:],
                                        start=first, stop=last,
                                        skip_group_check=True,
                                    )

                        # software-pipelined: PV runs two chunks behind the
                        # scores+exp so the PE never waits on the ACT exp.
                        pending = []
                        for c in range(nkc):
                            kd = c - 4 * qb
                            # diagonal chunks: only columns >= 128*kd live
                            lo = 128 * kd if kd > 0 else 0
                            tq_lo = slice(512 * qb + lo, 512 * (qb + 1))
                            scs = [psa.tile([128, 2, 512], F32, tag="pa",
                                            name="scA"),
                                   psa.tile([128, 2, 512], F32, tag="pa",
                                            name="scB")]
                            for j in range(4):
                                for half in range(2):
                                    cc = 2 * g + half
                                    nc.tensor.matmul(
                                        scs[j // 2][:, j % 2, lo:],
                                        khat[32 * j:32 * (j + 1), cc,
                                             128 * c:128 * (c + 1)],
                                        qhat[32 * j:32 * (j + 1), cc,
                                             tq_lo],
                                        start=(half == 0),
                                        stop=(half == 1),
                                        tile_position=(32 * j, 0),
                                    )
                            phs = []
                            for pj in range(2):
                                ph = tmp.tile([128, 2, 512], BF16, tag="ph",
                                              bufs=6, name=f"ph{pj}")
                                nc.scalar.activation(
                                    ph[:, :, lo:], scs[pj][:, :, lo:],
                                    ACTF.Exp)
                                if kd >= 0 and "M" not in sub:
                                    # diagonal boundary strip
                                    nc.vector.tensor_mul(
                                        ph[:, :, 128 * kd:128 * (kd + 1)],
                                        ph[:, :, 128 * kd:128 * (kd + 1)],
                                        mask_sb[:, None, :].to_broadcast(
                                            [128, 2, 128]),
                                    )
                                phs.append(ph)
                            pending.append((c, phs, lo, c == 0,
                                            c == nkc - 1))
                            if len(pending) > 2:
                                emit_pv(*pending.pop(0))
                        for p in pending:
                            emit_pv(*p)
                        # normalize: yhat rows = y / denom. Reciprocals of
                        # the two heads of a pair land on partitions 64/65;
                        # one K=2 matmul broadcasts both; staging copy on
                        # DVE so ACT stays exp-only.
                        rbss = []
                        for pj in range(2):
                            rcp2 = tmp.tile([66, 512], BF16, tag="rcp2",
                                            bufs=4)
                            with nc.allow_low_precision(
                                    reason="bf16 softmax div"):
                                # odd head first ([64:66], aligned start);
                                # even head then overwrites row 64
                                nc.vector.reciprocal(
                                    rcp2[64:66, :],
                                    ybank[2 * pj + 1][64:66, :])
                                nc.vector.reciprocal(
                                    rcp2[64:65, :], ybank[2 * pj][64:65, :])
                            rb = psa.tile([128, 2, 512], F32, tag="pa",
                                          name=f"rb{qb}_{g}_{pj}")
                            nc.tensor.matmul(
                                rb[:, 0, :],
                                selrb_sb[64:66, :],
                                rcp2[64:66, :],
                                start=True, stop=True,
                                tile_position=(64, 0),
                                skip_group_check=True,
                            )
                            rbs = tmp.tile([128, 512], BF16, tag="rbs",
                                           bufs=2)
                            nc.vector.tensor_copy(out=rbs, in_=rb[:, 0, :])
                            rbss.append(rbs)
                        for j in range(4):
                            hloc = 4 * g + j
                            nc.vector.tensor_mul(
                                yhat[64 * (hloc % 2):64 * (hloc % 2 + 1),
                                     hloc // 2, tqs],
                                ybank[j][0:64, :],
                                rbss[j // 2][64 * (j % 2):
                                             64 * (j % 2) + 64, :],
                            )

                    # ==== transposed out-projection for this tq block ====
                    # OUT^T[n, t] = sum_m Wo[m, n]^T yhat[m, t]
                    for n8 in range(NO8 if "C" in phases else 0):
                        op = psy.tile([128, 512], F32, tag="y",
                                      name=f"op{n8}_{qb}")
                        for m in range(4):
                            nc.tensor.matmul(
                                op,
                                wo_sb[:, m, 128 * n8:128 * (n8 + 1)],
                                yhat[:, m, tqs],
                                start=(m == 0), stop=(m == 3),
                            )
                        osb = tmp.tile([128, 512], BF16, tag="osb")
                        nc.vector.tensor_copy(out=osb, in_=op)
                        nc.sync.dma_start(
                            out=OUTT[128 * n8:128 * (n8 + 1), tqs],
                            in_=osb)

                # prefetch next pass's Wo during the tail
                if prefetch:
                    load_wo()

            # python-unrolled repeat: the Tile For_i back-edge costs ~600us
            # per iteration on this runtime, so unroll instead.
            load_qkv_weights()
            load_wo()
            for p in range(repeat):
                body(prefetch=(p < repeat - 1))

    nc.finalize()
    _unify_act_table_loads(nc)
    return nc


def _unify_act_table_loads(nc):
    """The table-set chooser greedily maps Exp -> exp_and_others and
    Ln -> natural_log, reloading tables (~2.7us each) at every Ln/Exp
    boundary. Every activation used here (Copy/Exp/Ln/Square) lives in
    natural_log_exp_and_others, so retarget the first load per block to
    that set and drop the rest."""
    from concourse.hw_specs import get_activation_tables

    need = {ACTF.Copy, ACTF.Exp, ACTF.Ln, ACTF.Square}
    tables = get_activation_tables(nc.m.arch)
    combined = None
    for idx, fns in enumerate(tables.values()):
        if need <= fns:
            combined = idx
            break
    assert combined is not None, "no table set covers Copy/Exp/Ln/Square"
    for b in nc.main_func.blocks:
        loads = [i for i in b.instructions
                 if isinstance(i, mybir.InstLoadActFuncSet)]
        if not loads:
            continue
        loads[0].act_func_set_id = combined
        drop = set(id(i) for i in loads[1:])
        b.instructions[:] = [i for i in b.instructions
                             if id(i) not in drop]


# ======================================================================
# Full-problem harness: 8 cores = 4 batch x 2 head-groups
# ======================================================================
B_FULL, T_FULL, C_FULL, H_FULL = 4, 2048, 1024, 16

_NC_CACHE = {}


def _get_nc(repeat=1, phases="ABC", sub="qkvrsm"):
    key = ("nc", repeat, phases, sub)
    if key not in _NC_CACHE:
        _NC_CACHE[key] = build_nc(T_FULL, C_FULL, C_FULL, repeat=repeat,
                                  phases=phases, sub=sub)
    return _NC_CACHE[key]


def make_in_maps(x, cos, sin, Wq, Wk, Wv, Wo):
    x, Wq, Wk, Wv, Wo = (np.asarray(a, dtype=np.float32)
                         for a in (x, Wq, Wk, Wv, Wo))
    cos_t = np.asarray(cos, dtype=np.float32)[0, 0]   # [T, 32]
    sin_t = np.asarray(sin, dtype=np.float32)[0, 0]
    consts = make_blob_consts(T_FULL, cos_t, sin_t)
    perm = qk_col_perm()
    KC = C_FULL // 128
    in_maps = []
    for core in range(8):
        b, hg = core // 2, core % 2
        cols = slice(512 * hg, 512 * (hg + 1))
        named = dict(consts)
        named["WQ"] = Wq[:, cols][:, perm].reshape(KC, 128, CH) \
            .transpose(1, 0, 2).reshape(128, KC * CH)
        named["WK"] = Wk[:, cols][:, perm].reshape(KC, 128, CH) \
            .transpose(1, 0, 2).reshape(128, KC * CH)
        named["WV"] = Wv[:, cols].reshape(KC, 128, CH) \
            .transpose(1, 0, 2).reshape(128, KC * CH)
        named["WO"] = Wo[cols, :].reshape(4, 128, C_FULL) \
            .transpose(1, 0, 2).reshape(128, 4 * C_FULL)
        blob = pack_blob(T_FULL, C_FULL, C_FULL, named)
        # [ki, tb, ko, tt] packing: row ki holds, per t-block, all ko
        # chunks' 512-t slices contiguously.
        xt = x[b].T.reshape(KC, 128, T_FULL // 512, 512)
        xt = xt.transpose(1, 2, 0, 3).reshape(128, KC * T_FULL)
        in_maps.append(dict(
            XT=np.ascontiguousarray(xt).astype(BFNP),
            CONST=blob,
        ))
    return in_maps


def gather_out(results):
    out = np.empty((B_FULL, T_FULL, C_FULL), dtype=np.float32)
    for b in range(B_FULL):
        s = (results[2 * b]["OUTT"].astype(np.float32)
             + results[2 * b + 1]["OUTT"].astype(np.float32))
        out[b] = s.T
    return out


def kernel(x, cos, sin, Wq, Wk, Wv, Wo):
    from concourse.bass_utils import run_bass_kernel_spmd
    nc = _get_nc()
    in_maps = make_in_maps(x, cos, sin, Wq, Wk, Wv, Wo)
    res = run_bass_kernel_spmd(nc, in_maps, core_ids=list(range(8)))
    return gather_out(res.results)
